# revision 2
# baseline (speedup 1.0000x reference)
"""IF spiking-neuron scan (charge / fire / hard-reset) on 8 Trainium2 cores.

Reference recurrence over t (elementwise on every [B, N] element):
    v = v + x_t
    s = (v - 1.0 >= 0)          # spike, 0.0/1.0
    v = (1 - s) * v             # hard reset to 0

Exact clamp reformulation used on device (state w):
    w' = min(x + w*[w < 1], 1)      # ONE custom DVE op (IF_STEP_ANT)
    spike s = [w' == 1]
A spike leaves w == 1.0 exactly (min clamp); the next step's mask
[w < 1] maps it to 0 so the charge u = x + 0 is bit-exact. u < 1 passes
through min() unchanged, so non-spiking steps are bit-exact too. Spikes
are recovered on the scalar engine as r = Sign(1 - w) -> uint8 in
{1, 0} (u > 1 gives Sign = -1 which saturates to 0): r == 1 - s.

The custom DVE op replaces the baseline's tensor_add +
scalar_tensor_tensor pair: one 3-operand pass per timestep instead of
two, halving the vector engine's operand traffic (the measured DVE
bottleneck). It is registered into concourse.dve_ops at import time
(same mechanism as the production ops; sha pinned at first lower()).

Sharding: pure data parallel over the B*N = 262144 element chains;
each core owns 32768 chains as [128 partitions x 256 lanes] with zero
communication. The host pre-transposes x to [P, T, F] per core so
every input DMA is a contiguous per-partition run (line-rate), feeds
timestep blocks through one HWDGE ring in consumption order, and
decodes spikes as s = 1 - r.
"""

import dataclasses
import re

import numpy as np

import concourse.tile as tile
from concourse import bacc, mybir, dve_ops
from concourse.bass_utils import run_bass_kernel_spmd
from concourse.dve_spec import Spec, Src0, Src1, C0, C1, Zero, minn, select
from concourse.dve_table_gen import dve_ver_for

T = 64
B = 32
N = 8192
NCORES = 8
PERCORE = (B * N) // NCORES  # 32768 chains per core
P = 128                      # SBUF partitions
F = PERCORE // P             # 256 chains per partition

V_TH = 1.0

f32 = mybir.dt.float32
u8 = mybir.dt.uint8

_NC_CACHE = {}


def register_if_step():
    """Register the fused IF-step custom DVE op (idempotent)."""
    name = "IF_STEP_ANT"
    for existing in dve_ops.OPS:
        if existing.name == name:
            return existing
    row = dve_ops._CUSTOM_DVE_ROW_BASE + len(dve_ops.OPS)
    assert row < 0x20, "custom-DVE opcode rows exhausted"
    dve_ops._SUB_OPCODE_FOR_NAME[name] = row
    op = dve_ops.DveOp(
        name,
        Spec(
            # out = min(in1 + in0*[in0 < s0], s1)
            body=minn(Src1 + select(Src0 < C0, Src0, Zero), C1),
            reference=lambda in0, in1, s0, s1, imm2: np.minimum(
                in1 + np.where(in0 < s0, in0, 0.0), s1
            ).astype(np.float32),
        ),
        subdim=False,
        uops_sha={},
    )
    ver = dve_ver_for("TRN2")
    try:
        op.compile(ver)
    except ValueError as e:
        # first lower() reports the uops sha; pin it and re-validate
        m = re.search(r"(v\d): ([0-9a-f]+) ", str(e))
        assert m, f"cannot parse uops sha from: {e}"
        op = dataclasses.replace(op, uops_sha={m.group(1): m.group(2)})
        dve_ops._COMPILE_CACHE.pop((name, ver), None)
    op.compile(ver)
    dve_ops.OPS.append(op)
    dve_ops.CUSTOM_DVE_SPECS[name] = op.spec
    return op


# timestep block sizes: small early blocks shrink the pipeline fill
# (per-block DMA-completion latency exposure), 8-step steady state
BLOCKS = [1, 1, 2, 2, 4, 4, 8, 8, 8, 8, 8, 8, 2]
assert sum(BLOCKS) == T


def build_nc(blocks=None):
    ifop = register_if_step()
    blocks = list(BLOCKS if blocks is None else blocks)
    nc = bacc.Bacc("TRN2", target_bir_lowering=False, debug=False)
    # host pre-transposed: [P, T, F]; per partition one contiguous T*F run
    x = nc.dram_tensor("x", [P, T, F], f32, kind="ExternalInput").ap()
    y = nc.dram_tensor("y", [P, T, F], u8, kind="ExternalOutput").ap()

    nblocks = len(blocks)
    with tile.TileContext(nc) as tc:
        with (
            tc.tile_pool(name="xin", bufs=nblocks) as xpool,
            tc.tile_pool(name="wb", bufs=6) as wpool,
            tc.tile_pool(name="sout", bufs=6) as spool,
            tc.tile_pool(name="vinit", bufs=1) as vpool,
        ):
            # prefetch every x block up-front on one ring, in consumption
            # order (concurrent rings share HBM bw and delay the head block)
            xts = []
            t0 = 0
            for bi, tb in enumerate(blocks):
                xt = xpool.tile([P, tb * F], f32, tag="xin", name=f"x_{bi}")
                if bi == 0 and tb > 1:
                    nc.sync.dma_start(xt[:, :F], x[:, 0:1, :])
                    nc.gpsimd.dma_start(xt[:, F:], x[:, 1:tb, :])
                else:
                    nc.sync.dma_start(xt[:], x[:, t0:t0 + tb, :])
                xts.append(xt)
                t0 += tb

            winit = vpool.tile([P, F], f32)
            nc.vector.memset(winit[:], 0.0)

            t0 = 0
            prev_w, prev_slot = None, 0
            for bi, tb in enumerate(blocks):
                xt = xts[bi]
                wh = wpool.tile([P, tb * F], f32, tag="wb", name=f"w_{bi}")
                for ti in range(tb):
                    if t0 + ti == 0:
                        src = winit[:]
                    elif ti == 0:
                        src = prev_w[:, prev_slot * F:(prev_slot + 1) * F]
                    else:
                        src = wh[:, (ti - 1) * F:ti * F]
                    nc.vector._custom_dve(
                        ifop,
                        out=wh[:, ti * F:(ti + 1) * F],
                        in0=src,
                        in1=xt[:, ti * F:(ti + 1) * F],
                        s0=V_TH, s1=V_TH,
                    )
                prev_w, prev_slot = wh, tb - 1
                # spike bytes: r = 1 - s  (1 if w < 1 else 0)
                st = spool.tile([P, tb * F], u8, tag="sout", name=f"s_{bi}")
                if bi == nblocks - 1:
                    # DVE is idle once the scan ends; computing the last
                    # block's spikes there removes the ACT tail
                    nc.vector.tensor_scalar(
                        st[:], wh[:], V_TH, None, mybir.AluOpType.is_lt
                    )
                else:
                    nc.scalar.activation(
                        st[:], wh[:], mybir.ActivationFunctionType.Sign,
                        bias=V_TH, scale=-1.0,
                    )
                nc.scalar.dma_start(y[:, t0:t0 + tb, :], st[:])
                t0 += tb
    nc.compile()
    return nc


def _get_nc():
    if "nc" not in _NC_CACHE:
        _NC_CACHE["nc"] = build_nc()
    return _NC_CACHE["nc"]


def _decode(res):
    out = np.empty((T, B * N), dtype=np.float32)
    for c in range(NCORES):
        yc = np.asarray(res.results[c]["y"])          # [P, T, F] uint8, r = 1-s
        r = yc.transpose(1, 0, 2).reshape(T, PERCORE)
        out[:, c * PERCORE:(c + 1) * PERCORE] = 1 - r
    return out.reshape(T, B, N)


def run_sharded(x_seq, trace=False, nc=None, **kwargs):
    if nc is None:
        nc = _get_nc()
    x2 = np.asarray(x_seq, dtype=np.float32).reshape(T, NCORES, P, F)
    in_maps = [
        {"x": np.ascontiguousarray(x2[:, c].transpose(1, 0, 2))}
        for c in range(NCORES)
    ]
    # A cold device occasionally fails its first execute; retry a few times.
    for attempt in range(3):
        try:
            res = run_bass_kernel_spmd(
                nc, in_maps, list(range(NCORES)), trace=trace, **kwargs
            )
            break
        except Exception:
            if attempt == 2:
                raise
            import time
            time.sleep(2.0)
    return _decode(res), res


def _spot_check(x_seq, out):
    """Exact numpy recurrence on a small sample of chains; guards against
    the (rare) cold-device execute that silently returns garbage."""
    xs = np.asarray(x_seq, dtype=np.float32).reshape(T, B * N)
    cols = np.arange(0, B * N, 4096)  # 64 chains spread across all cores
    v = np.zeros(len(cols), dtype=np.float32)
    ref = np.empty((T, len(cols)), dtype=np.float32)
    for t in range(T):
        v = v + xs[t, cols]
        s = (v >= np.float32(V_TH)).astype(np.float32)
        ref[t] = s
        v = (np.float32(1.0) - s) * v
    got = np.asarray(out).reshape(T, B * N)[:, cols]
    return np.array_equal(got, ref)


def kernel(x_seq):
    out, _ = run_sharded(x_seq)
    if not _spot_check(x_seq, out):
        out, _ = run_sharded(x_seq)  # one retry on a silently-bad execute
    return out
